# revision 34
# baseline (speedup 1.0000x reference)
"""Trainium2 Bass kernel for nn_Net_39230231281866 (dense_cnn).

Network: conv3x3(1->6) -> Taylor-sigmoid -> conv3x3(6->7) -> flatten
         -> fc(4032->128) -> sigmoid -> fc(128->10) -> log_softmax,
batch 8192, data-parallel over 8 NeuronCores (1024 samples/core).

Design (~53us vs ~99us v1 baseline):
  * conv1 as fp8 banded matmuls with split-K DoubleRow (each tile's
    K<=48 window rows folded to [Kh<=24, 2]), so every tile fits one
    32-row group and FOUR conv matmuls run concurrently in the PE array
    (tile_position 0/32/64/96). 35-tile cover of the 26x26 output.
  * Taylor-sigmoid replaced by ONE elementwise pass per conv tile,
    statically split between ScalarE (tanh, affine-fitted to the Taylor
    sigmoid) and VectorE (custom single-pass odd-quintic DVE op fitted
    likewise; per-partition bias via s0, c5 via the in1/C3 latch).
    Outputs are fp8e4 s-tiles, pair-packed for DoubleRow.
  * conv2+fc1 folded into W_comb, quantized fp8e4 with a global gain G
    that folds out through the fc1 sigmoid; contracted as 18 DoubleRow
    matmuls (K=256) per 512-sample slice.
  * The PE instruction order is pinned with order-only deps:
    [4-wide conv wall] then [two back-to-back z matmuls delayed by two
    quads] so z weights/moving data are always ready and streams chain.
  * An 8-matmul warm-up burst on zeroed scratch runs during the initial
    DMA wait to open the HAM clock gate before the real stream.
  * fc1 sigmoid computed as (sigma-0.5) via the same quintic DVE op
    (0.5 folded into fb2), so ScalarE's single exp/ln table re-load
    hides under the end of the main loop; fc2 in fp16.
  * DMA: one fp8 pre-windowed input slab per core in 6 chunked loads on
    the sync HWDGE ring; a single merged fp8 weight pack + consts on the
    scalar ring (each ring serializes ~2us completion receipts).
"""

import os
import numpy as np
import ml_dtypes

_B = 8192
_NCORES = 8
_PC = _B // _NCORES
_SLICE = 512
_NSL = _PC // _SLICE

_A_IN = 0.5          # conv matmul gain: cp = _A_IN * conv1(x)
_G = 64.0            # global fp8 gain for W_comb
_F8 = ml_dtypes.float8_e4m3

LAST_RESULTS = None


# ---------------- tiling ----------------

def _tiles():
    """35 rectangular tiles covering the 26x26 conv1 output, all with
    M = 6*noy*nox <= 128 and K = (noy+2)*(nox+2) <= 48 (2 fit in the PE
    rows as 64-row groups). Sorted by class so pairs share a class."""
    ts = []
    for oy0 in range(0, 24, 3):
        for (ox0, nox) in [(0, 7), (7, 7), (14, 7), (21, 5)]:
            ts.append((oy0, 3, ox0, nox))
    for (ox0, nox) in [(0, 10), (10, 10), (20, 6)]:
        ts.append((24, 2, ox0, nox))
    order = {(3, 7): 0, (3, 5): 1, (2, 10): 2, (2, 6): 3}
    ts.sort(key=lambda t: order[(t[1], t[3])])
    return ts


_TILES = _tiles()
_N_UNITS = 18            # 17 pairs + 1 single, per slice
_PAIR_CLASSES = []       # unit -> weight-matrix class index
_CLS_LIST = []
for _u in range(_N_UNITS):
    _ta = _TILES[2 * _u]
    _ca = (_ta[1], _ta[3])
    if 2 * _u + 1 < len(_TILES):
        _cb = (_TILES[2 * _u + 1][1], _TILES[2 * _u + 1][3])
    else:
        _cb = None
    if (_ca, _cb) not in _CLS_LIST:
        _CLS_LIST.append((_ca, _cb))
    _PAIR_CLASSES.append(_CLS_LIST.index((_ca, _cb)))

# tile -> engine: even tiles on ScalarE (tanh), odd on VectorE (quintic);
# tile 17 flipped to ScalarE for load balance (ACT is faster per op).
_TILE_ON_ACT = {ti: ((ti % 2 == 0) or ti == 17) and ti not in (32, 34)
                for ti in range(len(_TILES))}


# ---------------- fits (input-independent constants) ----------------

def _taylor_sig(x):
    t = -x
    return 1.0 / (2.0 + t * (1.0 + t * (0.5 + t * ((1.0 / 6.0) + t * (1.0 / 24.0)))))


def _fit_consts():
    # conv quintic: c5 u'^5 + c3 u'^3 + c1 u' + beta ~= taylor_sig(2u),
    # u' = u + dq, u = 0.5*(v + b1)
    u = np.linspace(-2.05, 2.05, 2051)
    w = np.exp(-0.5 * (u / 0.36) ** 2) + 3e-4
    tgt = _taylor_sig(2 * u)
    sw = np.sqrt(w)
    best = None
    for dq in np.linspace(-0.3, 0.3, 61):
        uu = u + dq
        A = np.stack([uu ** 5, uu ** 3, uu, np.ones_like(u)], 1)
        coef, *_ = np.linalg.lstsq(A * sw[:, None], tgt * sw, rcond=None)
        e = A @ coef - tgt
        L = (w * e ** 2).sum()
        if best is None or L < best[0]:
            best = (L, dq, coef)
    _, dq, (c5, c3, c1, q_beta) = best

    # tanh: a*tanh(g t + d) + b ~= taylor_sig(t), t = v + b1
    t = np.linspace(-2.6, 2.6, 2601)
    wt = np.exp(-0.5 * (t / 0.70) ** 2) + 3e-4
    tt = _taylor_sig(t)
    swt = np.sqrt(wt)
    best = None
    for g in np.linspace(0.4, 0.75, 36):
        for dd in np.linspace(-0.3, 0.3, 31):
            A = np.stack([np.tanh(g * t + dd), np.ones_like(t)], 1)
            coef, *_ = np.linalg.lstsq(A * swt[:, None], tt * swt, rcond=None)
            e = A @ coef - tt
            L = (wt * e ** 2).sum()
            if best is None or L < best[0]:
                best = (L, g, dd, coef)
    _, tg, td, (t_alpha, t_beta) = best

    # h quintic (odd): a5 y^5 + a3 y^3 + a1 y ~= sigmoid(y) - 0.5
    y = np.linspace(-1.2, 1.2, 1201)
    wy = np.exp(-0.5 * (y / 0.30) ** 2) + 1e-3
    ty = 1.0 / (1.0 + np.exp(-y)) - 0.5
    swy = np.sqrt(wy)
    A = np.stack([y ** 5, y ** 3, y], 1)
    (a5, a3, a1), *_ = np.linalg.lstsq(A * swy[:, None], ty * swy, rcond=None)
    return dict(dq=float(dq), c5=float(c5), c3=float(c3), c1=float(c1),
                q_beta=float(q_beta), tg=float(tg), td=float(td),
                t_alpha=float(t_alpha), t_beta=float(t_beta),
                a5=float(a5), a3=float(a3), a1=float(a1))


_FC = _fit_consts()


# ---------------- host prep ----------------

def _kh(cls):
    return ((cls[0] + 2) * (cls[1] + 2) + 1) // 2


# quads of 4 tiles -> 4 concurrent 32-row-group DoubleRow conv matmuls
_N_QUADS = (len(_TILES) + 3) // 4
_QUAD_CLASSES = []
_QCLS_LIST = []
for _q in range(_N_QUADS):
    _key = tuple((_TILES[i][1], _TILES[i][3]) if i < len(_TILES) else None
                 for i in range(4 * _q, 4 * _q + 4))
    if _key not in _QCLS_LIST:
        _QCLS_LIST.append(_key)
    _QUAD_CLASSES.append(_QCLS_LIST.index(_key))


def _host_prep(x, w1, b1, w2, b2, fw1, fb1, fw2, fb2):
    x = np.asarray(x, np.float32)
    w1 = np.asarray(w1, np.float32); b1 = np.asarray(b1, np.float32)
    w2 = np.asarray(w2, np.float32); b2 = np.asarray(b2, np.float32)
    fw1 = np.asarray(fw1, np.float32); fb1 = np.asarray(fb1, np.float32)
    fw2 = np.asarray(fw2, np.float32); fb2 = np.asarray(fb2, np.float32)
    F = _FC

    def _banded(cls):
        noy, nox = cls
        ky, kx = noy + 2, nox + 2
        wt = np.zeros((ky * kx, 128), np.float32)
        for oy in range(noy):
            for ox in range(nox):
                for oc in range(6):
                    m = (oy * nox + ox) * 6 + oc
                    for dy in range(3):
                        for dx in range(3):
                            wt[(oy + dy) * kx + (ox + dx), m] = \
                                _A_IN * w1[oc, 0, dy, dx]
        return wt

    # quad-class DoubleRow conv weights: tile i of a quad at partitions
    # 32i:32i+Kh, cols qc*256 + j*128 + f holds wt[j*Kh + p, f]
    w1pack = np.zeros((128, len(_QCLS_LIST) * 256), np.float32)
    for qc, key in enumerate(_QCLS_LIST):
        for i, cls in enumerate(key):
            if cls is None:
                continue
            wt = _banded(cls)
            K = wt.shape[0]
            kh = _kh(cls)
            wsplit = np.zeros((2 * kh, 128), np.float32)
            wsplit[:K] = wt
            for j in range(2):
                w1pack[32 * i:32 * i + kh,
                       qc * 256 + j * 128: qc * 256 + (j + 1) * 128] = \
                    wsplit[j * kh:(j + 1) * kh]

    # fold conv2 + fc1 -> Wc [128, 6*26*26], bias bcomb
    fw1r = fw1.reshape(128, 7, 24, 24)
    Wc = np.zeros((128, 6, 26, 26), np.float32)
    for dy in range(3):
        for dx in range(3):
            Wc[:, :, dy:dy + 24, dx:dx + 24] += np.einsum(
                "joyx,oi->jiyx", fw1r, w2[:, :, dy, dx], optimize=True)
    bcomb = fb1 + np.einsum("joyx,o->j", fw1r, b2)
    Wc_flat = Wc.reshape(128, 6 * 26 * 26)

    # wcpack [128, 18*256] fp8 (DoubleRow layout: unit u, j in {0,1} at
    # cols u*256 + j*128 + f), per-tile gain G*alpha; beta folds into bias
    wcpack = np.zeros((128, _N_UNITS * 256), np.float32)
    bc_eff = bcomb.copy()
    for ti, t in enumerate(_TILES):
        oy0, noy, ox0, nox = t
        M = noy * nox * 6
        unit, j = ti // 2, ti % 2
        alpha = F["t_alpha"] if _TILE_ON_ACT[ti] else 1.0
        beta = F["t_beta"] if _TILE_ON_ACT[ti] else F["q_beta"]
        cols = []
        for oy in range(noy):
            for ox in range(nox):
                for oc in range(6):
                    cols.append((oc * 26 + oy0 + oy) * 26 + ox0 + ox)
        Wt = Wc_flat[:, cols]                       # [128 feat, M]
        wcpack[0:M, unit * 256 + j * 128: unit * 256 + (j + 1) * 128] = \
            (_G * alpha) * Wt.T
        bc_eff += beta * Wt.sum(axis=1)

    bias_act = (F["tg"] * b1[np.arange(128) % 6] + F["td"]).astype(np.float32)
    bias_dve = (_A_IN * b1[np.arange(128) % 6] + F["dq"]).astype(np.float32)
    fb2e = fb2 + 0.5 * fw2.sum(axis=1)

    # all f32 per-partition consts packed into one [128, 45] DMA:
    # col 0 biasact, 1 biasdve, 2 c5, 3 a5/G^5, 4 G*bc_eff, 5:45 fb2r
    cpack = np.zeros((128, 45), np.float32)
    cpack[:, 0] = bias_act
    cpack[:, 1] = bias_dve
    cpack[:, 2] = F["c5"]
    cpack[:, 3] = F["a5"] / _G ** 5
    cpack[:, 4] = _G * bc_eff
    cpack[:, 5:45] = np.tile(fb2e.reshape(1, 10), (128, 4))

    consts = dict(
        wpack8=np.concatenate([w1pack.astype(_F8), wcpack.astype(_F8)],
                              axis=1),
        cpack=cpack,
        fw2t=np.ascontiguousarray(fw2.T).astype(np.float16),
    )

    # pre-windowed fp8 input slab per core: [128, nsl*nquads*1024].
    # Quad block (sl, q) at cols (sl*nq+q)*1024; tile i of the quad at
    # partitions 32i:32i+Kh with its split window rows as two 512-col
    # j-blocks (DoubleRow layout).
    x_pm = x.reshape(_B, 784).T.astype(_F8)         # [784, B]
    x_pm_pad = np.zeros((785, _B), _F8)             # row 784 stays zero pad
    x_pm_pad[:784] = x_pm
    slabs = []
    for c in range(_NCORES):
        slab = np.zeros((128, _NSL * _N_QUADS * 2 * _SLICE), _F8)
        for sl in range(_NSL):
            s0 = c * _PC + sl * _SLICE
            for ti, t in enumerate(_TILES):
                oy0, noy, ox0, nox = t
                ky, kx = noy + 2, nox + 2
                q, i = ti // 4, ti % 4
                kh = _kh((noy, nox))
                rows = ((np.arange(ky)[:, None] + oy0) * 28 +
                        (np.arange(kx)[None, :] + ox0)).reshape(-1)
                rows = np.concatenate(
                    [rows, np.full(2 * kh - ky * kx, 784, np.int64)])
                cb = (sl * _N_QUADS + q) * 2 * _SLICE
                for j in range(2):
                    slab[32 * i:32 * i + kh,
                         cb + j * _SLICE: cb + (j + 1) * _SLICE] = \
                        x_pm_pad[rows[j * kh:(j + 1) * kh], s0:s0 + _SLICE]
        slabs.append(slab)
    return slabs, consts


# ---------------- custom DVE op: odd quintic ----------------

def _register_sigpoly():
    import concourse.dve_ops as dve_ops
    if "SIGPOLY5_ANT" in dve_ops._SUB_OPCODE_FOR_NAME:
        return next(o for o in dve_ops.OPS if o.name == "SIGPOLY5_ANT")
    from concourse.dve_spec import (Spec, Src0, C0, C1, C2, C3, lower,
                                    _spill_c3_to_src1)
    from concourse.dve_uop import DveOpSpec

    # u = in0 + s0;  out = ((c5*u^2 + s1)*u^2 + imm2)*u   (c5 via in1)
    u = Src0 + C0
    w = u * u
    body = _spill_c3_to_src1(((C3 * w + C1) * w + C2) * u)

    def _ref(in0, in1, s0, s1, imm2):
        uu = in0.astype(np.float32) + s0
        ww = uu * uu
        c5 = np.asarray(in1, np.float32).reshape(in0.shape[0], -1)[:, :1]
        return ((c5 * ww + s1) * ww + imm2) * uu

    spec = Spec(body=body, reference=_ref)
    name = "SIGPOLY5_ANT"
    row = max(dve_ops._SUB_OPCODE_FOR_NAME.values()) + 1
    assert row < 0x20
    dve_ops._SUB_OPCODE_FOR_NAME[name] = row
    shas = {}
    for ver in ("v3", "v4"):
        tmp = DveOpSpec(name=name, opcode=row, uops=lower(spec, ver=ver),
                        rd1_en=True)
        shas[ver] = tmp.sha(ver)
    op = dve_ops.DveOp(name, spec, subdim=False, uops_sha=shas)
    dve_ops.OPS.append(op)
    dve_ops.CUSTOM_DVE_SPECS[name] = spec
    return op


def _pin_act_tables():
    """Pin Tanh -> exp_and_others, Exp/Ln -> natural_log_exp_and_others
    so the kernel costs exactly two ACT table loads."""
    import concourse.bacc as bacc
    import concourse.mybir as mybir
    if getattr(bacc, "_ant_tables_pinned", False):
        return
    orig = bacc.get_activation_tables
    AF = mybir.ActivationFunctionType

    def patched(arch):
        tabs = {k: set(v) for k, v in orig(arch).items()}
        for name, fns in tabs.items():
            if name != "exp_and_others":
                fns.discard(AF.Tanh)
            if name != "natural_log_exp_and_others":
                fns.discard(AF.Exp)
                fns.discard(AF.Ln)
        return tabs

    bacc.get_activation_tables = patched
    bacc._ant_tables_pinned = True


# ---------------- program ----------------

def _build_program():
    import concourse.bacc as bacc
    import concourse.mybir as mybir
    from concourse.tile import TileContext
    from concourse.tile_rust import add_dep_helper
    from concourse.alu_op_type import AluOpType

    f32 = mybir.dt.float32
    f16 = mybir.dt.float16
    f8 = mybir.dt.float8e4
    AF = mybir.ActivationFunctionType
    DR = mybir.MatmulPerfMode.DoubleRow
    sigpoly = _register_sigpoly()
    _pin_act_tables()
    F = _FC

    nc = bacc.Bacc()
    n_cols = _NSL * _N_QUADS * 2 * _SLICE
    w1_cols = len(_QCLS_LIST) * 256
    xwin_d = nc.declare_dram_parameter("xwin", [128, n_cols], f8, isOutput=False)
    wpack8_d = nc.declare_dram_parameter("wpack8",
                                         [128, w1_cols + _N_UNITS * 256], f8,
                                         isOutput=False)
    cpack_d = nc.declare_dram_parameter("cpack", [128, 45], f32, isOutput=False)
    fw2t_d = nc.declare_dram_parameter("fw2t", [128, 10], f16, isOutput=False)
    out_d = nc.declare_dram_parameter("out", [_PC, 10], f32, isOutput=True)

    # xwin chunk boundaries in cols (multiples of 1024 = one quad block);
    # small leading chunks so the first conv starts early
    ch_bounds = [0, 1024, 3072, 6144, 10240, 14336, n_cols]

    with TileContext(nc) as tc:
        with (
            tc.tile_pool(name="const", bufs=1) as cpool,
            tc.tile_pool(name="work", bufs=3) as wpool,
            tc.tile_pool(name="cps", bufs=6, space="PSUM") as cps,
            tc.tile_pool(name="zps", bufs=1, space="PSUM") as zps,
            tc.tile_pool(name="fps", bufs=1, space="PSUM") as fps,
        ):
            # scalar ring: merged fp8 weights, then f32 consts, then fw2t.
            # sync ring: only xwin chunks (+ output at the end).
            wpack8_sb = cpool.tile([128, w1_cols + _N_UNITS * 256], f8,
                                   tag="wp8", name="wpack8_sb", bufs=1)
            nc.scalar.dma_start(out=wpack8_sb, in_=wpack8_d[:])
            cpack_sb = cpool.tile([128, 45], f32, tag="cpk", name="cpack_sb",
                                  bufs=1)
            nc.scalar.dma_start(out=cpack_sb, in_=cpack_d[:])
            fw2t_sb = cpool.tile([128, 10], f16, tag="fw2t", name="fw2t_sb",
                                 bufs=1)
            nc.scalar.dma_start(out=fw2t_sb, in_=fw2t_d[:])
            xw = []          # (tile, col0, ncols) per chunk
            for j in range(len(ch_bounds) - 1):
                c0, c1 = ch_bounds[j], ch_bounds[j + 1]
                t = cpool.tile([128, c1 - c0], f8, tag=f"xw{j}",
                               name=f"xw{j}", bufs=1)
                xw.append((t, c0, c1 - c0))
                nc.sync.dma_start(out=t, in_=xwin_d[:, c0:c1])
            biasact_sb = cpack_sb[:, 0:1]
            biasdve_sb = cpack_sb[:, 1:2]
            cvec_sb = cpack_sb[:, 2:4]
            bch_sb = cpack_sb[:, 4:5]
            fb2r_sb = cpack_sb[:, 5:45]

            def _xw_ap(cb, width):
                for t, c0, nc_ in xw:
                    if c0 <= cb and cb + width <= c0 + nc_:
                        return t, cb - c0
                raise AssertionError("chunk boundary crosses quad block")

            tanh_insts = []
            zs = []
            pe_chain = [None]

            def _pe(inst):
                if pe_chain[0] is not None:
                    add_dep_helper(inst.ins, pe_chain[0].ins, sync=False,
                                   reason="pe order")
                pe_chain[0] = inst
                return inst

            # HAM warm-up: dummy matmuls on a zeroed scratch tile keep the
            # PE busy during the initial DMA wait so the clock gate opens
            # (1.2 -> 2.4 GHz) before the real stream begins.
            warm_sb = cpool.tile([128, _SLICE], f8, tag="warm",
                                 name="warm_sb", bufs=1)
            nc.gpsimd.memset(warm_sb, 0.0)
            warm_ps = zps.tile([128, _SLICE], f32, tag="z", name="warm_ps")
            for _w in range(6):
                _pe(nc.tensor.matmul(
                    warm_ps, warm_sb[:, 0:128], warm_sb,
                    start=True, stop=True, skip_group_check=True))

            for sl in range(_NSL):
                z = zps.tile([128, _SLICE], f32, tag="z", name=f"z{sl}")
                zs.append(z)
                s_tiles = {}
                pend_z = []

                def _emit_z(u, z=z):
                    s = s_tiles.pop(u)
                    single = (2 * u + 1 >= len(_TILES))
                    if single:
                        _pe(nc.tensor.matmul(
                            z, wpack8_sb[:, w1_cols + u * 256:w1_cols + u * 256 + 128],
                            s[:, 0:_SLICE], start=(u == 0),
                            stop=(u == _N_UNITS - 1)))
                    else:
                        _pe(nc.tensor.matmul(
                            z,
                            wpack8_sb[:, w1_cols + u * 256:w1_cols + (u + 1) * 256].rearrange(
                                "p (j f) -> p j f", j=2),
                            s.rearrange("p (j n) -> p j n", j=2),
                            start=(u == 0), stop=(u == _N_UNITS - 1),
                            perf_mode=DR))

                for q in range(_N_QUADS):
                    qc = _QUAD_CLASSES[q]
                    cb = (sl * _N_QUADS + q) * 2 * _SLICE
                    xwt, lo = _xw_ap(cb, 2 * _SLICE)
                    ntiles = min(4, len(_TILES) - 4 * q)
                    # conv wall: up to 4 concurrent DoubleRow MMs
                    cpt = []
                    for i in range(ntiles):
                        t = _TILES[4 * q + i]
                        kh = _kh((t[1], t[3]))
                        cp = cps.tile([128, _SLICE], f32, tag="cp",
                                      name=f"cp{sl}_{4 * q + i}")
                        cpt.append(cp)
                        _pe(nc.tensor.matmul(
                            cp,
                            wpack8_sb[32 * i:32 * i + kh,
                                      qc * 256:(qc + 1) * 256].rearrange(
                                          "p (j f) -> p j f", j=2),
                            xwt[32 * i:32 * i + kh,
                                lo:lo + 2 * _SLICE].rearrange(
                                    "p (j n) -> p j n", j=2),
                            start=True, stop=True, perf_mode=DR,
                            tile_position=(32 * i, 0)))
                    # nonlinearity per tile, pair-shaped s output
                    for h in range((ntiles + 1) // 2):
                        u = 2 * q + h
                        s = wpool.tile([128, 2 * _SLICE], f8, tag="s",
                                       name=f"s{sl}_{u}", bufs=8)
                        s_tiles[u] = s
                        for jj in range(min(2, ntiles - 2 * h)):
                            ti = 4 * q + 2 * h + jj
                            cp = cpt[2 * h + jj]
                            dst = s[:, jj * _SLICE:(jj + 1) * _SLICE]
                            if _TILE_ON_ACT[ti]:
                                ti_ = nc.scalar.activation(
                                    dst, cp, AF.Tanh,
                                    bias=biasact_sb[:], scale=F["tg"] / _A_IN)
                                tanh_insts.append(ti_)
                            else:
                                nc.vector._custom_dve(
                                    sigpoly, out=dst, in0=cp,
                                    in1=cvec_sb[:, 0:1], s0=biasdve_sb[:],
                                    s1=F["c3"], imm2=F["c1"])
                        pend_z.append(u)
                    # z delayed by two quads so s is always ready
                    while len(pend_z) > 4:
                        _emit_z(pend_z.pop(0))
                while pend_z:
                    _emit_z(pend_z.pop(0))

            # ---- tail: h = (sigma-0.5) via quintic, fc2, log_softmax ----
            last_tanh = tanh_insts[-1]
            for sl in range(_NSL):
                hp = wpool.tile([128, _SLICE], f16, tag="h", name=f"h{sl}")
                nc.vector._custom_dve(
                    sigpoly, out=hp, in0=zs[sl], in1=cvec_sb[:, 1:2],
                    s0=bch_sb[:], s1=F["a3"] / _G ** 3, imm2=F["a1"] / _G)
                ng = _SLICE // 128
                fp = fps.tile([128, 10 * ng], f32, tag="fp", name=f"fp{sl}",
                              bufs=1)
                for g in range(ng):
                    nc.tensor.matmul(fp[:, g * 10:(g + 1) * 10],
                                     hp[:, g * 128:(g + 1) * 128],
                                     fw2t_sb[:], start=True, stop=True)
                lg = wpool.tile([128, 10 * ng], f32, tag="lg", name=f"lg{sl}")
                nc.vector.tensor_tensor(out=lg, in0=fp, in1=fb2r_sb[:, 0:10 * ng],
                                        op=AluOpType.add)
                e = wpool.tile([128, 10 * ng], f32, tag="e", name=f"e{sl}")
                ei = nc.scalar.activation(e, lg, AF.Exp)
                add_dep_helper(ei.ins, last_tanh.ins, sync=False,
                               reason="exp after last tanh (table sets)")
                ssum = wpool.tile([128, ng], f32, tag="ss", name=f"ss{sl}")
                nc.vector.tensor_reduce(
                    ssum, e.rearrange("p (g k) -> p g k", k=10),
                    axis=mybir.AxisListType.X, op=AluOpType.add)
                lns = wpool.tile([128, ng], f32, tag="ls", name=f"ls{sl}")
                li = nc.scalar.activation(lns, ssum, AF.Ln)
                add_dep_helper(li.ins, last_tanh.ins, sync=False,
                               reason="ln after last tanh (table sets)")
                ot = wpool.tile([128, 10 * ng], f32, tag="ot", name=f"ot{sl}")
                nc.vector.tensor_tensor(
                    out=ot.rearrange("p (g k) -> p g k", k=10),
                    in0=lg.rearrange("p (g k) -> p g k", k=10),
                    in1=lns.rearrange("p (g o) -> p g o", o=1).broadcast_to(
                        [128, ng, 10]),
                    op=AluOpType.subtract)
                orow = sl * _SLICE
                nc.sync.dma_start(
                    out=out_d[orow:orow + _SLICE, :].rearrange(
                        "(g p) k -> p g k", p=128),
                    in_=ot.rearrange("p (g k) -> p g k", k=10))
    nc.compile()
    return nc


_PROGRAM_CACHE = {}


def kernel(x, w1, b1, w2, b2, fw1, fb1, fw2, fb2):
    global LAST_RESULTS
    slabs, consts = _host_prep(x, w1, b1, w2, b2, fw1, fb1, fw2, fb2)

    if "nc" not in _PROGRAM_CACHE:
        _PROGRAM_CACHE["nc"] = _build_program()
    nc = _PROGRAM_CACHE["nc"]

    in_maps = []
    for c in range(_NCORES):
        m = dict(consts)
        m["xwin"] = slabs[c]
        in_maps.append(m)

    from concourse.bass_utils import run_bass_kernel_spmd
    trace = bool(int(os.environ.get("BASS_KERNEL_TRACE", "0")))
    res = run_bass_kernel_spmd(nc, in_maps, core_ids=list(range(_NCORES)),
                               trace=trace)
    LAST_RESULTS = res
    return np.concatenate([r["out"] for r in res.results], axis=0)


# revision 35
# speedup vs baseline: 1.0132x; 1.0132x over previous
"""Trainium2 Bass kernel for nn_Net_39230231281866 (dense_cnn).

Network: conv3x3(1->6) -> Taylor-sigmoid -> conv3x3(6->7) -> flatten
         -> fc(4032->128) -> sigmoid -> fc(128->10) -> log_softmax,
batch 8192, data-parallel over 8 NeuronCores (1024 samples/core).

Design (~53us vs ~99us v1 baseline):
  * conv1 as fp8 banded matmuls with split-K DoubleRow (each tile's
    K<=48 window rows folded to [Kh<=24, 2]), so every tile fits one
    32-row group and FOUR conv matmuls run concurrently in the PE array
    (tile_position 0/32/64/96). 35-tile cover of the 26x26 output.
  * Taylor-sigmoid replaced by ONE elementwise pass per conv tile,
    statically split between ScalarE (tanh, affine-fitted to the Taylor
    sigmoid) and VectorE (custom single-pass odd-quintic DVE op fitted
    likewise; per-partition bias via s0, c5 via the in1/C3 latch).
    Outputs are fp8e4 s-tiles, pair-packed for DoubleRow.
  * conv2+fc1 folded into W_comb, quantized fp8e4 with a global gain G
    that folds out through the fc1 sigmoid; contracted as 18 DoubleRow
    matmuls (K=256) per 512-sample slice.
  * The PE instruction order is pinned with order-only deps:
    [4-wide conv wall] then [two back-to-back z matmuls delayed by two
    quads] so z weights/moving data are always ready and streams chain.
  * A 6-matmul warm-up burst on zeroed scratch runs during the initial
    DMA wait to open the HAM clock gate before the real stream.
  * fc1 sigmoid computed as (sigma-0.5) via the same quintic DVE op
    (0.5 folded into fb2), so ScalarE's single exp/ln table re-load
    hides under the end of the main loop; fc2 in fp16.
  * DMA: one fp8 pre-windowed input slab per core in 6 chunked loads on
    the sync HWDGE ring; a single merged fp8 weight pack + consts on the
    scalar ring (each ring serializes ~2us completion receipts).
"""

import os
import numpy as np
import ml_dtypes

_B = 8192
_NCORES = 8
_PC = _B // _NCORES
_SLICE = 512
_NSL = _PC // _SLICE

_A_IN = 0.5          # conv matmul gain: cp = _A_IN * conv1(x)
_G = 64.0            # global fp8 gain for W_comb
_F8 = ml_dtypes.float8_e4m3

LAST_RESULTS = None


# ---------------- tiling ----------------

def _tiles():
    """35 rectangular tiles covering the 26x26 conv1 output, all with
    M = 6*noy*nox <= 128 and K = (noy+2)*(nox+2) <= 48 (2 fit in the PE
    rows as 64-row groups). Sorted by class so pairs share a class."""
    ts = []
    for oy0 in range(0, 24, 3):
        for (ox0, nox) in [(0, 7), (7, 7), (14, 7), (21, 5)]:
            ts.append((oy0, 3, ox0, nox))
    for (ox0, nox) in [(0, 10), (10, 10), (20, 6)]:
        ts.append((24, 2, ox0, nox))
    order = {(3, 7): 0, (3, 5): 1, (2, 10): 2, (2, 6): 3}
    ts.sort(key=lambda t: order[(t[1], t[3])])
    return ts


_TILES = _tiles()
_N_UNITS = 18            # 17 pairs + 1 single, per slice
_PAIR_CLASSES = []       # unit -> weight-matrix class index
_CLS_LIST = []
for _u in range(_N_UNITS):
    _ta = _TILES[2 * _u]
    _ca = (_ta[1], _ta[3])
    if 2 * _u + 1 < len(_TILES):
        _cb = (_TILES[2 * _u + 1][1], _TILES[2 * _u + 1][3])
    else:
        _cb = None
    if (_ca, _cb) not in _CLS_LIST:
        _CLS_LIST.append((_ca, _cb))
    _PAIR_CLASSES.append(_CLS_LIST.index((_ca, _cb)))

# tile -> engine: even tiles on ScalarE (tanh), odd on VectorE (quintic);
# tile 17 flipped to ScalarE for load balance (ACT is faster per op).
_TILE_ON_ACT = {ti: ((ti % 2 == 0) or ti == 17) and ti not in (32, 34)
                for ti in range(len(_TILES))}


# ---------------- fits (input-independent constants) ----------------

def _taylor_sig(x):
    t = -x
    return 1.0 / (2.0 + t * (1.0 + t * (0.5 + t * ((1.0 / 6.0) + t * (1.0 / 24.0)))))


def _fit_consts():
    # conv quintic: c5 u'^5 + c3 u'^3 + c1 u' + beta ~= taylor_sig(2u),
    # u' = u + dq, u = 0.5*(v + b1)
    u = np.linspace(-2.05, 2.05, 2051)
    w = np.exp(-0.5 * (u / 0.36) ** 2) + 3e-4
    tgt = _taylor_sig(2 * u)
    sw = np.sqrt(w)
    best = None
    for dq in np.linspace(-0.3, 0.3, 61):
        uu = u + dq
        A = np.stack([uu ** 5, uu ** 3, uu, np.ones_like(u)], 1)
        coef, *_ = np.linalg.lstsq(A * sw[:, None], tgt * sw, rcond=None)
        e = A @ coef - tgt
        L = (w * e ** 2).sum()
        if best is None or L < best[0]:
            best = (L, dq, coef)
    _, dq, (c5, c3, c1, q_beta) = best

    # tanh: a*tanh(g t + d) + b ~= taylor_sig(t), t = v + b1
    t = np.linspace(-2.6, 2.6, 2601)
    wt = np.exp(-0.5 * (t / 0.70) ** 2) + 3e-4
    tt = _taylor_sig(t)
    swt = np.sqrt(wt)
    best = None
    for g in np.linspace(0.4, 0.75, 36):
        for dd in np.linspace(-0.3, 0.3, 31):
            A = np.stack([np.tanh(g * t + dd), np.ones_like(t)], 1)
            coef, *_ = np.linalg.lstsq(A * swt[:, None], tt * swt, rcond=None)
            e = A @ coef - tt
            L = (wt * e ** 2).sum()
            if best is None or L < best[0]:
                best = (L, g, dd, coef)
    _, tg, td, (t_alpha, t_beta) = best

    # h quintic (odd): a5 y^5 + a3 y^3 + a1 y ~= sigmoid(y) - 0.5
    y = np.linspace(-1.2, 1.2, 1201)
    wy = np.exp(-0.5 * (y / 0.30) ** 2) + 1e-3
    ty = 1.0 / (1.0 + np.exp(-y)) - 0.5
    swy = np.sqrt(wy)
    A = np.stack([y ** 5, y ** 3, y], 1)
    (a5, a3, a1), *_ = np.linalg.lstsq(A * swy[:, None], ty * swy, rcond=None)
    return dict(dq=float(dq), c5=float(c5), c3=float(c3), c1=float(c1),
                q_beta=float(q_beta), tg=float(tg), td=float(td),
                t_alpha=float(t_alpha), t_beta=float(t_beta),
                a5=float(a5), a3=float(a3), a1=float(a1))


_FC = _fit_consts()


# ---------------- host prep ----------------

def _kh(cls):
    return ((cls[0] + 2) * (cls[1] + 2) + 1) // 2


# quads of 4 tiles -> 4 concurrent 32-row-group DoubleRow conv matmuls
_N_QUADS = (len(_TILES) + 3) // 4
_QUAD_CLASSES = []
_QCLS_LIST = []
for _q in range(_N_QUADS):
    _key = tuple((_TILES[i][1], _TILES[i][3]) if i < len(_TILES) else None
                 for i in range(4 * _q, 4 * _q + 4))
    if _key not in _QCLS_LIST:
        _QCLS_LIST.append(_key)
    _QUAD_CLASSES.append(_QCLS_LIST.index(_key))


def _host_prep(x, w1, b1, w2, b2, fw1, fb1, fw2, fb2):
    x = np.asarray(x, np.float32)
    w1 = np.asarray(w1, np.float32); b1 = np.asarray(b1, np.float32)
    w2 = np.asarray(w2, np.float32); b2 = np.asarray(b2, np.float32)
    fw1 = np.asarray(fw1, np.float32); fb1 = np.asarray(fb1, np.float32)
    fw2 = np.asarray(fw2, np.float32); fb2 = np.asarray(fb2, np.float32)
    F = _FC

    def _banded(cls):
        noy, nox = cls
        ky, kx = noy + 2, nox + 2
        wt = np.zeros((ky * kx, 128), np.float32)
        for oy in range(noy):
            for ox in range(nox):
                for oc in range(6):
                    m = (oy * nox + ox) * 6 + oc
                    for dy in range(3):
                        for dx in range(3):
                            wt[(oy + dy) * kx + (ox + dx), m] = \
                                _A_IN * w1[oc, 0, dy, dx]
        return wt

    # quad-class DoubleRow conv weights: tile i of a quad at partitions
    # 32i:32i+Kh, cols qc*256 + j*128 + f holds wt[j*Kh + p, f]
    w1pack = np.zeros((128, len(_QCLS_LIST) * 256), np.float32)
    for qc, key in enumerate(_QCLS_LIST):
        for i, cls in enumerate(key):
            if cls is None:
                continue
            wt = _banded(cls)
            K = wt.shape[0]
            kh = _kh(cls)
            wsplit = np.zeros((2 * kh, 128), np.float32)
            wsplit[:K] = wt
            for j in range(2):
                w1pack[32 * i:32 * i + kh,
                       qc * 256 + j * 128: qc * 256 + (j + 1) * 128] = \
                    wsplit[j * kh:(j + 1) * kh]

    # fold conv2 + fc1 -> Wc [128, 6*26*26], bias bcomb
    fw1r = fw1.reshape(128, 7, 24, 24)
    Wc = np.zeros((128, 6, 26, 26), np.float32)
    for dy in range(3):
        for dx in range(3):
            Wc[:, :, dy:dy + 24, dx:dx + 24] += np.einsum(
                "joyx,oi->jiyx", fw1r, w2[:, :, dy, dx], optimize=True)
    bcomb = fb1 + np.einsum("joyx,o->j", fw1r, b2)
    Wc_flat = Wc.reshape(128, 6 * 26 * 26)

    # wcpack [128, 18*256] fp8 (DoubleRow layout: unit u, j in {0,1} at
    # cols u*256 + j*128 + f), per-tile gain G*alpha; beta folds into bias
    wcpack = np.zeros((128, _N_UNITS * 256), np.float32)
    bc_eff = bcomb.copy()
    for ti, t in enumerate(_TILES):
        oy0, noy, ox0, nox = t
        M = noy * nox * 6
        unit, j = ti // 2, ti % 2
        alpha = F["t_alpha"] if _TILE_ON_ACT[ti] else 1.0
        beta = F["t_beta"] if _TILE_ON_ACT[ti] else F["q_beta"]
        cols = []
        for oy in range(noy):
            for ox in range(nox):
                for oc in range(6):
                    cols.append((oc * 26 + oy0 + oy) * 26 + ox0 + ox)
        Wt = Wc_flat[:, cols]                       # [128 feat, M]
        wcpack[0:M, unit * 256 + j * 128: unit * 256 + (j + 1) * 128] = \
            (_G * alpha) * Wt.T
        bc_eff += beta * Wt.sum(axis=1)

    bias_act = (F["tg"] * b1[np.arange(128) % 6] + F["td"]).astype(np.float32)
    bias_dve = (_A_IN * b1[np.arange(128) % 6] + F["dq"]).astype(np.float32)
    fb2e = fb2 + 0.5 * fw2.sum(axis=1)

    # all f32 per-partition consts packed into one [128, 45] DMA:
    # col 0 biasact, 1 biasdve, 2 c5, 3 a5/G^5, 4 G*bc_eff, 5:45 fb2r
    cpack = np.zeros((128, 45), np.float32)
    cpack[:, 0] = bias_act
    cpack[:, 1] = bias_dve
    cpack[:, 2] = F["c5"]
    cpack[:, 3] = F["a5"] / _G ** 5
    cpack[:, 4] = _G * bc_eff
    cpack[:, 5:45] = np.tile(fb2e.reshape(1, 10), (128, 4))

    consts = dict(
        wpack8=np.concatenate([w1pack.astype(_F8), wcpack.astype(_F8)],
                              axis=1),
        cpack=cpack,
        fw2t=np.ascontiguousarray(fw2.T).astype(np.float16),
    )

    # pre-windowed fp8 input slab per core: [128, nsl*nquads*1024].
    # Quad block (sl, q) at cols (sl*nq+q)*1024; tile i of the quad at
    # partitions 32i:32i+Kh with its split window rows as two 512-col
    # j-blocks (DoubleRow layout).
    x_pm = x.reshape(_B, 784).T.astype(_F8)         # [784, B]
    x_pm_pad = np.zeros((785, _B), _F8)             # row 784 stays zero pad
    x_pm_pad[:784] = x_pm
    slabs = []
    for c in range(_NCORES):
        slab = np.zeros((128, _NSL * _N_QUADS * 2 * _SLICE), _F8)
        for sl in range(_NSL):
            s0 = c * _PC + sl * _SLICE
            for ti, t in enumerate(_TILES):
                oy0, noy, ox0, nox = t
                ky, kx = noy + 2, nox + 2
                q, i = ti // 4, ti % 4
                kh = _kh((noy, nox))
                rows = ((np.arange(ky)[:, None] + oy0) * 28 +
                        (np.arange(kx)[None, :] + ox0)).reshape(-1)
                rows = np.concatenate(
                    [rows, np.full(2 * kh - ky * kx, 784, np.int64)])
                cb = (sl * _N_QUADS + q) * 2 * _SLICE
                for j in range(2):
                    slab[32 * i:32 * i + kh,
                         cb + j * _SLICE: cb + (j + 1) * _SLICE] = \
                        x_pm_pad[rows[j * kh:(j + 1) * kh], s0:s0 + _SLICE]
        slabs.append(slab)
    return slabs, consts


# ---------------- custom DVE op: odd quintic ----------------

def _register_sigpoly():
    import concourse.dve_ops as dve_ops
    if "SIGPOLY5_ANT" in dve_ops._SUB_OPCODE_FOR_NAME:
        return next(o for o in dve_ops.OPS if o.name == "SIGPOLY5_ANT")
    from concourse.dve_spec import (Spec, Src0, C0, C1, C2, C3, lower,
                                    _spill_c3_to_src1)
    from concourse.dve_uop import DveOpSpec

    # u = in0 + s0;  out = ((c5*u^2 + s1)*u^2 + imm2)*u   (c5 via in1)
    u = Src0 + C0
    w = u * u
    body = _spill_c3_to_src1(((C3 * w + C1) * w + C2) * u)

    def _ref(in0, in1, s0, s1, imm2):
        uu = in0.astype(np.float32) + s0
        ww = uu * uu
        c5 = np.asarray(in1, np.float32).reshape(in0.shape[0], -1)[:, :1]
        return ((c5 * ww + s1) * ww + imm2) * uu

    spec = Spec(body=body, reference=_ref)
    name = "SIGPOLY5_ANT"
    row = max(dve_ops._SUB_OPCODE_FOR_NAME.values()) + 1
    assert row < 0x20
    dve_ops._SUB_OPCODE_FOR_NAME[name] = row
    shas = {}
    for ver in ("v3", "v4"):
        tmp = DveOpSpec(name=name, opcode=row, uops=lower(spec, ver=ver),
                        rd1_en=True)
        shas[ver] = tmp.sha(ver)
    op = dve_ops.DveOp(name, spec, subdim=False, uops_sha=shas)
    dve_ops.OPS.append(op)
    dve_ops.CUSTOM_DVE_SPECS[name] = spec
    return op


def _pin_act_tables():
    """Pin Tanh -> exp_and_others, Exp/Ln -> natural_log_exp_and_others
    so the kernel costs exactly two ACT table loads."""
    import concourse.bacc as bacc
    import concourse.mybir as mybir
    if getattr(bacc, "_ant_tables_pinned", False):
        return
    orig = bacc.get_activation_tables
    AF = mybir.ActivationFunctionType

    def patched(arch):
        tabs = {k: set(v) for k, v in orig(arch).items()}
        for name, fns in tabs.items():
            if name != "exp_and_others":
                fns.discard(AF.Tanh)
            if name != "natural_log_exp_and_others":
                fns.discard(AF.Exp)
                fns.discard(AF.Ln)
        return tabs

    bacc.get_activation_tables = patched
    bacc._ant_tables_pinned = True


# ---------------- program ----------------

def _build_program():
    import concourse.bacc as bacc
    import concourse.mybir as mybir
    from concourse.tile import TileContext
    from concourse.tile_rust import add_dep_helper
    from concourse.alu_op_type import AluOpType

    f32 = mybir.dt.float32
    f16 = mybir.dt.float16
    f8 = mybir.dt.float8e4
    AF = mybir.ActivationFunctionType
    DR = mybir.MatmulPerfMode.DoubleRow
    sigpoly = _register_sigpoly()
    _pin_act_tables()
    F = _FC

    nc = bacc.Bacc()
    n_cols = _NSL * _N_QUADS * 2 * _SLICE
    w1_cols = len(_QCLS_LIST) * 256
    xwin_d = nc.declare_dram_parameter("xwin", [128, n_cols], f8, isOutput=False)
    wpack8_d = nc.declare_dram_parameter("wpack8",
                                         [128, w1_cols + _N_UNITS * 256], f8,
                                         isOutput=False)
    cpack_d = nc.declare_dram_parameter("cpack", [128, 45], f32, isOutput=False)
    fw2t_d = nc.declare_dram_parameter("fw2t", [128, 10], f16, isOutput=False)
    out_d = nc.declare_dram_parameter("out", [_PC, 10], f32, isOutput=True)

    # xwin chunk boundaries in cols (multiples of 1024 = one quad block);
    # small leading chunks so the first conv starts early
    ch_bounds = [0, 1024, 3072, 6144, 10240, 14336, n_cols]

    with TileContext(nc) as tc:
        with (
            tc.tile_pool(name="const", bufs=1) as cpool,
            tc.tile_pool(name="work", bufs=3) as wpool,
            tc.tile_pool(name="cps", bufs=6, space="PSUM") as cps,
            tc.tile_pool(name="zps", bufs=1, space="PSUM") as zps,
            tc.tile_pool(name="fps", bufs=1, space="PSUM") as fps,
        ):
            # scalar ring: merged fp8 weights, then f32 consts, then fw2t.
            # sync ring: only xwin chunks (+ output at the end).
            wpack8_sb = cpool.tile([128, w1_cols + _N_UNITS * 256], f8,
                                   tag="wp8", name="wpack8_sb", bufs=1)
            nc.scalar.dma_start(out=wpack8_sb, in_=wpack8_d[:])
            cpack_sb = cpool.tile([128, 45], f32, tag="cpk", name="cpack_sb",
                                  bufs=1)
            nc.scalar.dma_start(out=cpack_sb, in_=cpack_d[:])
            fw2t_sb = cpool.tile([128, 10], f16, tag="fw2t", name="fw2t_sb",
                                 bufs=1)
            nc.scalar.dma_start(out=fw2t_sb, in_=fw2t_d[:])
            xw = []          # (tile, col0, ncols) per chunk
            for j in range(len(ch_bounds) - 1):
                c0, c1 = ch_bounds[j], ch_bounds[j + 1]
                t = cpool.tile([128, c1 - c0], f8, tag=f"xw{j}",
                               name=f"xw{j}", bufs=1)
                xw.append((t, c0, c1 - c0))
                nc.sync.dma_start(out=t, in_=xwin_d[:, c0:c1])
            biasact_sb = cpack_sb[:, 0:1]
            biasdve_sb = cpack_sb[:, 1:2]
            cvec_sb = cpack_sb[:, 2:4]
            bch_sb = cpack_sb[:, 4:5]
            fb2r_sb = cpack_sb[:, 5:45]

            def _xw_ap(cb, width):
                for t, c0, nc_ in xw:
                    if c0 <= cb and cb + width <= c0 + nc_:
                        return t, cb - c0
                raise AssertionError("chunk boundary crosses quad block")

            tanh_insts = []
            zs = []
            pe_chain = [None]

            def _pe(inst):
                if pe_chain[0] is not None:
                    add_dep_helper(inst.ins, pe_chain[0].ins, sync=False,
                                   reason="pe order")
                pe_chain[0] = inst
                return inst

            # HAM warm-up: dummy matmuls on a zeroed scratch tile keep the
            # PE busy during the initial DMA wait so the clock gate opens
            # (1.2 -> 2.4 GHz) before the real stream begins.
            warm_sb = cpool.tile([128, _SLICE], f8, tag="warm",
                                 name="warm_sb", bufs=1)
            nc.gpsimd.memset(warm_sb, 0.0)
            warm_ps = zps.tile([128, _SLICE], f32, tag="z", name="warm_ps")
            for _w in range(6):
                _pe(nc.tensor.matmul(
                    warm_ps, warm_sb[:, 0:128], warm_sb,
                    start=True, stop=True, skip_group_check=True))

            for sl in range(_NSL):
                z = zps.tile([128, _SLICE], f32, tag="z", name=f"z{sl}")
                zs.append(z)
                s_tiles = {}
                pend_z = []

                def _emit_z(u, z=z):
                    s = s_tiles.pop(u)
                    single = (2 * u + 1 >= len(_TILES))
                    if single:
                        _pe(nc.tensor.matmul(
                            z, wpack8_sb[:, w1_cols + u * 256:w1_cols + u * 256 + 128],
                            s[:, 0:_SLICE], start=(u == 0),
                            stop=(u == _N_UNITS - 1)))
                    else:
                        _pe(nc.tensor.matmul(
                            z,
                            wpack8_sb[:, w1_cols + u * 256:w1_cols + (u + 1) * 256].rearrange(
                                "p (j f) -> p j f", j=2),
                            s.rearrange("p (j n) -> p j n", j=2),
                            start=(u == 0), stop=(u == _N_UNITS - 1),
                            perf_mode=DR))

                for q in range(_N_QUADS):
                    qc = _QUAD_CLASSES[q]
                    cb = (sl * _N_QUADS + q) * 2 * _SLICE
                    xwt, lo = _xw_ap(cb, 2 * _SLICE)
                    ntiles = min(4, len(_TILES) - 4 * q)
                    # conv wall: up to 4 concurrent DoubleRow MMs
                    cpt = []
                    for i in range(ntiles):
                        t = _TILES[4 * q + i]
                        kh = _kh((t[1], t[3]))
                        cp = cps.tile([128, _SLICE], f32, tag="cp",
                                      name=f"cp{sl}_{4 * q + i}")
                        cpt.append(cp)
                        _pe(nc.tensor.matmul(
                            cp,
                            wpack8_sb[32 * i:32 * i + kh,
                                      qc * 256:(qc + 1) * 256].rearrange(
                                          "p (j f) -> p j f", j=2),
                            xwt[32 * i:32 * i + kh,
                                lo:lo + 2 * _SLICE].rearrange(
                                    "p (j n) -> p j n", j=2),
                            start=True, stop=True, perf_mode=DR,
                            tile_position=(32 * i, 0)))
                    # nonlinearity per tile, pair-shaped s output
                    for h in range((ntiles + 1) // 2):
                        u = 2 * q + h
                        s = wpool.tile([128, 2 * _SLICE], f8, tag="s",
                                       name=f"s{sl}_{u}", bufs=8)
                        s_tiles[u] = s
                        for jj in range(min(2, ntiles - 2 * h)):
                            ti = 4 * q + 2 * h + jj
                            cp = cpt[2 * h + jj]
                            dst = s[:, jj * _SLICE:(jj + 1) * _SLICE]
                            if _TILE_ON_ACT[ti]:
                                ti_ = nc.scalar.activation(
                                    dst, cp, AF.Tanh,
                                    bias=biasact_sb[:], scale=F["tg"] / _A_IN)
                                tanh_insts.append(ti_)
                            else:
                                nc.vector._custom_dve(
                                    sigpoly, out=dst, in0=cp,
                                    in1=cvec_sb[:, 0:1], s0=biasdve_sb[:],
                                    s1=F["c3"], imm2=F["c1"])
                        pend_z.append(u)
                    # z delayed by two quads so s is always ready
                    while len(pend_z) > 4:
                        _emit_z(pend_z.pop(0))
                while pend_z:
                    _emit_z(pend_z.pop(0))

            # ---- tail: h = (sigma-0.5) via quintic, fc2, log_softmax ----
            last_tanh = tanh_insts[-1]
            for sl in range(_NSL):
                hp = wpool.tile([128, _SLICE], f16, tag="h", name=f"h{sl}")
                nc.vector._custom_dve(
                    sigpoly, out=hp, in0=zs[sl], in1=cvec_sb[:, 1:2],
                    s0=bch_sb[:], s1=F["a3"] / _G ** 3, imm2=F["a1"] / _G)
                ng = _SLICE // 128
                fp = fps.tile([128, 10 * ng], f32, tag="fp", name=f"fp{sl}",
                              bufs=1)
                for g in range(ng):
                    nc.tensor.matmul(fp[:, g * 10:(g + 1) * 10],
                                     hp[:, g * 128:(g + 1) * 128],
                                     fw2t_sb[:], start=True, stop=True)
                lg = wpool.tile([128, 10 * ng], f32, tag="lg", name=f"lg{sl}")
                nc.vector.tensor_tensor(out=lg, in0=fp, in1=fb2r_sb[:, 0:10 * ng],
                                        op=AluOpType.add)
                e = wpool.tile([128, 10 * ng], f32, tag="e", name=f"e{sl}")
                ei = nc.scalar.activation(e, lg, AF.Exp)
                add_dep_helper(ei.ins, last_tanh.ins, sync=False,
                               reason="exp after last tanh (table sets)")
                ssum = wpool.tile([128, ng], f32, tag="ss", name=f"ss{sl}")
                nc.vector.tensor_reduce(
                    ssum, e.rearrange("p (g k) -> p g k", k=10),
                    axis=mybir.AxisListType.X, op=AluOpType.add)
                lns = wpool.tile([128, ng], f32, tag="ls", name=f"ls{sl}")
                li = nc.scalar.activation(lns, ssum, AF.Ln)
                add_dep_helper(li.ins, last_tanh.ins, sync=False,
                               reason="ln after last tanh (table sets)")
                ot = wpool.tile([128, 10 * ng], f32, tag="ot", name=f"ot{sl}")
                nc.vector.tensor_tensor(
                    out=ot.rearrange("p (g k) -> p g k", k=10),
                    in0=lg.rearrange("p (g k) -> p g k", k=10),
                    in1=lns.rearrange("p (g o) -> p g o", o=1).broadcast_to(
                        [128, ng, 10]),
                    op=AluOpType.subtract)
                orow = sl * _SLICE
                nc.sync.dma_start(
                    out=out_d[orow:orow + _SLICE, :].rearrange(
                        "(g p) k -> p g k", p=128),
                    in_=ot.rearrange("p (g k) -> p g k", k=10))
    nc.compile()
    return nc


_PROGRAM_CACHE = {}


def kernel(x, w1, b1, w2, b2, fw1, fb1, fw2, fb2):
    global LAST_RESULTS
    slabs, consts = _host_prep(x, w1, b1, w2, b2, fw1, fb1, fw2, fb2)

    if "nc" not in _PROGRAM_CACHE:
        _PROGRAM_CACHE["nc"] = _build_program()
    nc = _PROGRAM_CACHE["nc"]

    in_maps = []
    for c in range(_NCORES):
        m = dict(consts)
        m["xwin"] = slabs[c]
        in_maps.append(m)

    from concourse.bass_utils import run_bass_kernel_spmd
    trace = bool(int(os.environ.get("BASS_KERNEL_TRACE", "0")))
    res = run_bass_kernel_spmd(nc, in_maps, core_ids=list(range(_NCORES)),
                               trace=trace)
    LAST_RESULTS = res
    return np.concatenate([r["out"] for r in res.results], axis=0)
